# revision 2
# baseline (speedup 1.0000x reference)
"""Trainium2 Bass kernel for the 2-layer GCN (GCNConv+ReLU+BN x2, mean-pool).

Strategy (8 NeuronCores, SPMD, dest-node sharded):
- 256-byte gather rows: dims 0-122 fp16 + dims 123-132 fp8(e4m3), halving
  random-gather HBM traffic (adds ~1e-3 rel err, gate is 2e-2).
  GCN_EW=256 keeps the exact 512B fp16 layout (DTAIL=0) for A/B testing.
- Ragged chunk counts: per-(block,half) T = ceil(max-over-cores count / 128)
  instead of the global max, cutting padding in the E/matmul/gather work.
- Merged gathers: one dma_gather per (super-block, half) instead of per
  (block, half, 8-chunk sub), amortizing the ~1us fixed SWDGE cost.
- Engine rebalance: one-hot E builds split DVE/Pool, epilogue copies+relu on
  the Activation engine.
"""
import os
import numpy as np
import ml_dtypes
from contextlib import ExitStack

import concourse.bacc as bacc
import concourse.bass as bass
import concourse.mybir as mybir
import concourse.tile as tile
from concourse.library_config import mlp
from concourse.bass_utils import run_bass_kernel_spmd

dt = mybir.dt
NCORES = 8
PB = 128            # dest nodes per block
EPS = 1e-5
G_OUT = 2048

EW = int(os.environ.get("GCN_EW", "256"))      # fp16 elems per gather row
DTAIL = 10 if EW == 128 else 0                  # dims stored as fp8
SBK = int(os.environ.get("GCN_SBK", "4"))       # blocks per gather unit
GP_NUM = int(os.environ.get("GCN_GPNUM", "0"))  # of every 4 E builds on Pool
GT_BUFS = int(os.environ.get("GCN_GTBUFS", "4"))
NO_ACT = bool(int(os.environ.get("GCN_NOACT", "0")))
GCAP = int(os.environ.get("GCN_GCAP", "8"))  # max chunks per dma_gather
SP = bool(int(os.environ.get("GCN_SP", "1")))  # single_packet
AGC = int(os.environ.get("GCN_AGC", "4"))  # AllGather chunks

F8 = ml_dtypes.float8_e4m3


def pack_rows(h):
    """[n, 133] float -> [n, EW] fp16-viewed packed rows."""
    n, D = h.shape
    dmain = D - DTAIL
    out = np.zeros((n, EW * 2), dtype=np.uint8)
    out[:, : 2 * dmain] = (
        np.ascontiguousarray(h[:, :dmain]).astype(np.float16).view(np.uint8)
    )
    if DTAIL:
        out[:, 2 * dmain : 2 * dmain + DTAIL] = (
            np.ascontiguousarray(h[:, dmain:]).astype(F8).view(np.uint8)
        )
    return out.view(np.float16)


# ---------------------------------------------------------------- host prep
def preprocess(x, edge_index, batch):
    N, D = x.shape
    G = G_OUT
    NB = -(-N // (NCORES * PB))
    S = NB * PB
    TOT = NCORES * S
    L = TOT // 2
    assert L <= 32768 and TOT - L <= 32768

    r = np.asarray(edge_index[0], dtype=np.int64)
    c = np.asarray(edge_index[1], dtype=np.int64)
    loops = np.arange(N, dtype=np.int64)
    r = np.concatenate([r, loops])
    c = np.concatenate([c, loops])
    deg = np.bincount(c, minlength=N).astype(np.float64)
    dinv = np.where(deg > 0, 1.0 / np.sqrt(deg), 0.0)
    w = (dinv[r] * dinv[c]).astype(np.float32)

    # position permutation: node n lives at row pos(n) of xbuf/h1f so that
    # chunked AllGathers write contiguous h1f ranges. AGC=1 -> identity.
    Sc = S // AGC
    assert S % AGC == 0
    def pos(n):
        k, rr = n // S, n % S
        ch = rr // Sc
        return ch * (NCORES * Sc) + k * Sc + (rr % Sc)
    rp = pos(r)
    half = (rp >= L).astype(np.int64)
    src = np.where(half == 0, rp, rp - L).astype(np.int64)
    blk = c // PB                    # global block 0..NCORES*NB-1
    d = (c % PB).astype(np.int64)

    ngroups = NCORES * NB * 2
    gid = blk * 2 + half
    order = np.lexsort((src, gid))
    src, d, w, gid = src[order], d[order], w[order], gid[order]
    counts = np.bincount(gid, minlength=ngroups)
    starts = np.concatenate([[0], np.cumsum(counts)])

    # SPMD-uniform ragged T: per (local block, half), max count across cores
    cmat = counts.reshape(NCORES, NB, 2)
    Tu = -(-cmat.max(axis=0) // PB)          # [NB, 2]
    Tu = np.maximum(Tu, 0).astype(np.int64)

    # unit layout: per unit u (SBK blocks), per half: chunks for each block
    nunits = -(-NB // SBK)
    units = []
    ncol = 0
    for u in range(nunits):
        b0, b1 = u * SBK, min((u + 1) * SBK, NB)
        halves = []
        for h in (0, 1):
            offs = {}
            o = 0
            for b in range(b0, b1):
                offs[b] = o
                o += int(Tu[b, h])
            halves.append(dict(h=h, nch=o, col0=ncol, offs=offs))
            ncol += o
        units.append(dict(b0=b0, b1=b1, halves=halves))
    NCH = ncol                                  # total chunks per core/layer

    idx_arr = np.zeros((NCORES, 128, 8 * NCH), dtype=np.int16)
    d_arr = np.zeros((NCORES, 128, NCH), dtype=np.float32)
    w_arr = np.zeros((NCORES, 128, NCH), dtype=np.float32)
    for k in range(NCORES):
        for un in units:
            for hd in un["halves"]:
                h = hd["h"]
                if hd["nch"] == 0:
                    continue
                sp_ = np.zeros(hd["nch"] * PB, dtype=np.int16)
                dp_ = np.zeros(hd["nch"] * PB, dtype=np.float32)
                wp_ = np.zeros(hd["nch"] * PB, dtype=np.float32)
                for b in range(un["b0"], un["b1"]):
                    g = (k * NB + b) * 2 + h
                    n = counts[g]
                    o = hd["offs"][b] * PB
                    sp_[o : o + n] = src[starts[g] : starts[g] + n]
                    dp_[o : o + n] = d[starts[g] : starts[g] + n]
                    wp_[o : o + n] = w[starts[g] : starts[g] + n]
                nch = hd["nch"]
                wrapped = sp_.reshape(8 * nch, 16).T        # [16, 8*nch]
                idx_arr[k, :, 8 * hd["col0"] : 8 * (hd["col0"] + nch)] = (
                    np.tile(wrapped, (8, 1))
                )
                d_arr[k, :, hd["col0"] : hd["col0"] + nch] = dp_.reshape(
                    nch, PB
                ).T
                w_arr[k, :, hd["col0"] : hd["col0"] + nch] = wp_.reshape(
                    nch, PB
                ).T

    # pooling metadata (same as v1)
    batch_pad = np.full(TOT, -1, dtype=np.int64)
    batch_pad[:N] = np.asarray(batch)
    blocks = batch_pad.reshape(NCORES * NB, PB)
    valid = blocks >= 0
    base = np.where(
        valid.any(axis=1),
        np.where(valid, blocks, np.iinfo(np.int64).max).min(axis=1),
        0,
    )
    bloc = np.where(valid, blocks - base[:, None], -1).astype(np.float32)
    bloc_arr = bloc.reshape(NCORES, NB, PB).transpose(0, 2, 1).copy()

    cnts = np.bincount(np.asarray(batch), minlength=G).astype(np.float32)
    posv = pos(np.arange(TOT))
    return dict(N=N, D=D, G=G, NB=NB, S=S, TOT=TOT, L=L, pos=posv,
                units=units, NCH=NCH, Tu=Tu,
                idx_arr=idx_arr, d_arr=d_arr, w_arr=w_arr,
                bloc_arr=bloc_arr, base=base, cnts=cnts)


def fold_bn(g, beta, rm, rv):
    gp = (np.asarray(g) / np.sqrt(np.asarray(rv) + EPS)).astype(np.float32)
    bp = (np.asarray(beta) - np.asarray(rm) * gp).astype(np.float32)
    return gp, bp


# ---------------------------------------------------------------- bass build
def build_nc(pp, reps=1, single_core=False):
    f16, f32, i16 = dt.float16, dt.float32, dt.int16
    f8, u8 = dt.float8e4, dt.uint8
    D, NB, S, TOT, L = pp["D"], pp["NB"], pp["S"], pp["TOT"], pp["L"]
    NCH = pp["NCH"]
    units = pp["units"]
    DLO = D - 128
    DMAIN = D - DTAIL
    NCHMAX = max(hd["nch"] for un in units for hd in un["halves"])
    nc = bacc.Bacc("TRN2", target_bir_lowering=False, debug=False,
                   num_devices=1 if single_core else NCORES)

    xbuf = nc.dram_tensor("xbuf", [TOT, EW], f16, kind="ExternalInput")
    idxt = nc.dram_tensor("idxt", [128, 8 * NCH], i16, kind="ExternalInput")
    dcol = nc.dram_tensor("dcol", [128, NCH], f32, kind="ExternalInput")
    wcol = nc.dram_tensor("wcol", [128, NCH], f32, kind="ExternalInput")
    bcol = nc.dram_tensor("bcol", [128, NB], f32, kind="ExternalInput")
    iot = nc.dram_tensor("iot", [128, 128], f16, kind="ExternalInput")
    idn = nc.dram_tensor("idn", [128, 128], f32, kind="ExternalInput")
    onesr = nc.dram_tensor("onesr", [1, 128], f16, kind="ExternalInput")
    whi = nc.dram_tensor("whi", [2, 128, D], f16, kind="ExternalInput")
    wlo = nc.dram_tensor("wlo", [2, DLO, D], f16, kind="ExternalInput")
    brow = nc.dram_tensor("brow", [2, 1, D], f16, kind="ExternalInput")
    gam = nc.dram_tensor("gam", [2, 128, D], f32, kind="ExternalInput")
    bet = nc.dram_tensor("bet", [2, 128, D], f32, kind="ExternalInput")
    outp = nc.dram_tensor("outp", [NB * PB, D], f32, kind="ExternalOutput")
    h1sl = nc.dram_tensor("h1sl", [S, EW], f16, kind="Internal")
    h1f = nc.dram_tensor("h1f", [TOT, EW], f16, kind="Internal",
                         addr_space="Shared")

    with tile.TileContext(nc) as tc, ExitStack() as ctx:
        cp = ctx.enter_context(tc.tile_pool(name="consts", bufs=1))
        gp_ = ctx.enter_context(tc.tile_pool(name="gath", bufs=GT_BUFS))
        tp_ = ctx.enter_context(tc.tile_pool(name="tails", bufs=GT_BUFS))
        ep = ctx.enter_context(tc.tile_pool(name="onehot", bufs=8))
        sp = ctx.enter_context(tc.tile_pool(name="stage", bufs=3))
        pagg = ctx.enter_context(tc.tile_pool(name="pagg", bufs=2, space="PSUM"))
        if DTAIL:
            pagt = ctx.enter_context(tc.tile_pool(name="pagt", bufs=2, space="PSUM"))
            ptr = ctx.enter_context(tc.tile_pool(name="ptr", bufs=1, space="PSUM"))
            ppool = ctx.enter_context(tc.tile_pool(name="ppool", bufs=1, space="PSUM"))
        else:
            pagt = None
            ptr = ctx.enter_context(tc.tile_pool(name="ptr", bufs=2, space="PSUM"))
            ppool = ctx.enter_context(tc.tile_pool(name="ppool", bufs=2, space="PSUM"))
        ph = ctx.enter_context(tc.tile_pool(name="ph", bufs=2, space="PSUM"))

        def load_const(name, dram, shape, dtype):
            t = cp.tile(shape, dtype, name=name)
            nc.sync.dma_start(t[:], dram)
            return t

        idxS = load_const("idxS", idxt[:, :], [128, 8 * NCH], i16)
        dS = load_const("dS", dcol[:, :], [128, NCH], f32)
        wS = load_const("wS", wcol[:, :], [128, NCH], f32)
        bS = load_const("bS", bcol[:, :], [128, NB], f32)
        iotS = load_const("iotS", iot[:, :], [128, 128], f16)
        idnS = load_const("idnS", idn[:, :], [128, 128], f32)
        onesS = load_const("onesS", onesr[:, :], [1, 128], f16)
        whiS = [load_const(f"whiS{l}", whi[l, :, :], [128, D], f16) for l in range(2)]
        wloS = [load_const(f"wloS{l}", wlo[l, :, :], [DLO, D], f16) for l in range(2)]
        browS = [load_const(f"browS{l}", brow[l, :, :], [1, D], f16) for l in range(2)]
        gamS = [load_const(f"gamS{l}", gam[l, :, :], [128, D], f32) for l in range(2)]
        betS = [load_const(f"betS{l}", bet[l, :, :], [128, D], f32) for l in range(2)]

        nc.gpsimd.load_library(mlp)

        env = dict(locals())
        for rep in range(reps):
            env["rep"] = rep
            _do_body(nc, tc, env)

    nc.compile()
    return nc


def _do_body(nc, tc, env):
    (D, NB, S, TOT, L, NCH, NCHMAX, units, rep, single_core, DMAIN) = (
        env[k] for k in ("D", "NB", "S", "TOT", "L", "NCH", "NCHMAX", "units",
                         "rep", "single_core", "DMAIN"))
    (xbuf, h1f, h1sl, outp) = (env[k] for k in ("xbuf", "h1f", "h1sl", "outp"))
    (gp_, tp_, ep, sp, pagg, pagt, ptr, ph, ppool) = (env[k] for k in
        ("gp_", "tp_", "ep", "sp", "pagg", "pagt", "ptr", "ph", "ppool"))
    (idxS, dS, wS, bS, iotS, idnS, onesS) = (env[k] for k in
        ("idxS", "dS", "wS", "bS", "iotS", "idnS", "onesS"))
    (whiS, wloS, browS, gamS, betS) = (env[k] for k in
        ("whiS", "wloS", "browS", "gamS", "betS"))
    f16, f32 = dt.float16, dt.float32
    f8, u8 = dt.float8e4, dt.uint8
    DLO = D - 128
    R = rep
    ecnt = 0

    for layer in range(2):
        src = xbuf if layer == 0 else h1f
        for ui, un in enumerate(units):
            # --- gathers for this unit (both halves)
            gts = {}
            tts = {}
            for hd in un["halves"]:
                h, nch = hd["h"], hd["nch"]
                if nch == 0:
                    continue
                gt = gp_.tile([128, NCHMAX, EW], f16,
                              name=f"gt_{R}_{layer}_{ui}_{h}", tag="gt")
                in_ap = src[0:TOT, :] if h == 0 else src[L:TOT, :]
                for o in range(0, nch, GCAP):
                    sub = min(GCAP, nch - o)
                    nc.gpsimd.dma_gather(
                        gt[:, o : o + sub, :], in_ap,
                        idxS[:, 8 * (hd["col0"] + o) : 8 * (hd["col0"] + o + sub)],
                        sub * PB, sub * PB, EW, single_packet=SP)
                gts[h] = gt
                if DTAIL:
                    tt = tp_.tile([128, NCHMAX, DTAIL], f16,
                                  name=f"tt_{R}_{layer}_{ui}_{h}", tag="tt")
                    gt8 = gt[:, 0:nch, :].bitcast(f8)
                    cpeng = nc.vector if NO_ACT else nc.scalar
                    (cpeng.tensor_copy if NO_ACT else cpeng.copy)(
                        tt[:, 0:nch, :], gt8[:, :, 2 * DMAIN : 2 * DMAIN + DTAIL])
                    tts[h] = tt

            # --- per block: one-hot matmul chain + epilogue
            for b in range(un["b0"], un["b1"]):
                chunks = []          # (half, slot, col)
                for hd in un["halves"]:
                    h = hd["h"]
                    o = hd["offs"][b]
                    nxt = hd["offs"].get(b + 1, hd["nch"])
                    for t in range(o, nxt):
                        chunks.append((h, t, hd["col0"] + t))
                agg = pagg.tile([128, DMAIN], f32, name=f"agg_{R}_{layer}_{b}",
                                tag="agg")
                if DTAIL:
                    agt = pagt.tile([128, 16], f32,
                                    name=f"agt_{R}_{layer}_{b}", tag="agt")
                nchk = len(chunks)
                for ci, (h, slot, cc) in enumerate(chunks):
                    eng = nc.gpsimd if (ci % 4 < GP_NUM) else nc.vector
                    E = ep.tile([128, 128], f16, name=f"E_{R}_{layer}_{cc}",
                                tag="E")
                    eng.tensor_scalar(
                        E[:], iotS[:], dS[:, cc : cc + 1], wS[:, cc : cc + 1],
                        op0=mybir.AluOpType.is_equal,
                        op1=mybir.AluOpType.mult)
                    nc.tensor.matmul(
                        agg[:], E[:], gts[h][:, slot, 0:DMAIN],
                        start=(ci == 0), stop=(ci == nchk - 1))
                    if DTAIL:
                        nc.tensor.matmul(
                            agt[:, 0:DTAIL], E[:],
                            tts[h][:, slot, :],
                            start=(ci == 0), stop=(ci == nchk - 1))
                aggS = sp.tile([128, D], f32, name=f"aggS_{R}_{layer}_{b}",
                               tag="aggS")
                if nchk == 0:
                    nc.vector.memset(aggS[:], 0)
                else:
                    if NO_ACT:
                        nc.vector.tensor_copy(aggS[:, 0:DMAIN], agg[:])
                        if DTAIL:
                            nc.vector.tensor_copy(aggS[:, DMAIN:D], agt[:, 0:DTAIL])
                    else:
                        nc.scalar.copy(aggS[:, 0:DMAIN], agg[:])
                        if DTAIL:
                            nc.scalar.copy(aggS[:, DMAIN:D], agt[:, 0:DTAIL])
                psT = ptr.tile([128, 256], f32, name=f"psT_{R}_{layer}_{b}",
                               tag="psT")
                nc.tensor.transpose(psT[:, 0:128], aggS[:, 0:128], idnS[:])
                nc.tensor.transpose(psT[0:DLO, 128:256], aggS[:, 128:D], idnS[:])
                t1 = sp.tile([128, 128], f16, name=f"t1_{R}_{layer}_{b}", tag="t1")
                t2 = sp.tile([DLO, 128], f16, name=f"t2_{R}_{layer}_{b}", tag="t2")
                if NO_ACT:
                    nc.vector.tensor_copy(t1[:], psT[:, 0:128])
                    nc.vector.tensor_copy(t2[:], psT[0:DLO, 128:256])
                else:
                    nc.scalar.copy(t1[:], psT[:, 0:128])
                    nc.scalar.copy(t2[:], psT[0:DLO, 128:256])
                zps = ph.tile([128, D], f32, name=f"zps_{R}_{layer}_{b}", tag="zps")
                nc.tensor.matmul(zps[:], t1[:], whiS[layer][:],
                                 start=True, stop=False)
                nc.tensor.matmul(zps[:], t2[:], wloS[layer][:],
                                 start=False, stop=False)
                nc.tensor.matmul(zps[:], onesS[:], browS[layer][:],
                                 start=False, stop=True)
                rl = sp.tile([128, D], f32, name=f"rl_{R}_{layer}_{b}", tag="rl")
                nc.scalar.activation(rl[:], zps[:],
                                     mybir.ActivationFunctionType.Relu)
                m1 = sp.tile([128, D], f32, name=f"m1_{R}_{layer}_{b}", tag="m1")
                nc.vector.tensor_mul(m1[:], rl[:], gamS[layer][:])
                if layer == 0:
                    # pack to 256B rows: fp16 main + fp8 tail
                    hS = sp.tile([128, EW], f16, name=f"hS_{R}_{layer}_{b}",
                                 tag="hS")
                    nc.vector.tensor_add(hS[:, 0:DMAIN], m1[:, 0:DMAIN],
                                         betS[layer][:, 0:DMAIN])
                    if DTAIL:
                        t8 = sp.tile([128, 16], f8, name=f"t8_{R}_{b}", tag="t8")
                        nc.vector.tensor_add(t8[:, 0:DTAIL], m1[:, DMAIN:D],
                                             betS[layer][:, DMAIN:D])
                        nc.vector.tensor_copy(
                            hS[:].bitcast(u8)[:, 2 * DMAIN : 2 * DMAIN + DTAIL],
                            t8[:].bitcast(u8)[:, 0:DTAIL])
                    nc.sync.dma_start(h1sl[b * PB:(b + 1) * PB, :], hS[:])
                    Sc = S // AGC
                    for cch in range(AGC):
                        lastb = ((cch + 1) * Sc - 1) // PB
                        if b == lastb:
                            r0, r1 = cch * Sc, (cch + 1) * Sc
                            if single_core or os.environ.get("GCN_NOCOLL"):
                                nc.gpsimd.dma_start(
                                    h1f[NCORES * r0 : NCORES * r0 + (r1 - r0), :],
                                    h1sl[r0:r1, :])
                            else:
                                nc.gpsimd.collective_compute(
                                    "AllGather", mybir.AluOpType.bypass,
                                    replica_groups=[list(range(NCORES))],
                                    ins=[h1sl[r0:r1, :].opt()],
                                    outs=[h1f[NCORES * r0 : NCORES * r1, :].opt()])
                else:
                    hS = sp.tile([128, D], f16, name=f"hS_{R}_{layer}_{b}",
                                 tag="hS")
                    nc.vector.tensor_add(hS[:], m1[:], betS[layer][:])
                    P = ep.tile([128, 128], f16, name=f"P_{R}_{b}", tag="E")
                    nc.vector.tensor_scalar(
                        P[:], iotS[:], bS[:, b : b + 1], None,
                        op0=mybir.AluOpType.is_equal)
                    pps = ppool.tile([128, D], f32, name=f"pps_{R}_{b}",
                                     tag="pps")
                    nc.tensor.matmul(pps[:], P[:], hS[:], start=True, stop=True)
                    po = sp.tile([128, D], f32, name=f"po_{R}_{b}", tag="po")
                    if NO_ACT:
                        nc.vector.tensor_copy(po[:], pps[:])
                    else:
                        nc.scalar.copy(po[:], pps[:])
                    nc.sync.dma_start(outp[b * PB:(b + 1) * PB, :], po[:])



# ---------------------------------------------------------------- entry
_NC_CACHE = {}


def prepare(x, edge_index, batch, W1, b1, W2, b2,
            g1, beta1, rm1, rv1, g2, beta2, rm2, rv2, reps=1):
    x = np.asarray(x, dtype=np.float32)
    pp = preprocess(x, np.asarray(edge_index), np.asarray(batch))
    D = pp["D"]
    key = (pp["NCH"], reps)
    if key not in _NC_CACHE:
        _NC_CACHE[key] = build_nc(pp, reps=reps)
    nc = _NC_CACHE[key]

    xb = np.zeros((pp["TOT"], EW), dtype=np.float16)
    xb[pp["pos"][: pp["N"]]] = pack_rows(x)
    iotv = np.broadcast_to(np.arange(128, dtype=np.float16), (128, 128)).copy()
    idnv = np.eye(128, dtype=np.float32)
    onesv = np.ones((1, 128), dtype=np.float16)
    g1p, b1p = fold_bn(g1, beta1, rm1, rv1)
    g2p, b2p = fold_bn(g2, beta2, rm2, rv2)
    W1 = np.asarray(W1); W2 = np.asarray(W2)
    b1 = np.asarray(b1); b2 = np.asarray(b2)
    whiv = np.stack([W1[:128], W2[:128]]).astype(np.float16)
    wlov = np.stack([W1[128:], W2[128:]]).astype(np.float16)
    browv = np.stack([b1[None, :], b2[None, :]]).astype(np.float16)
    gamv = np.stack([np.broadcast_to(g1p, (128, D)),
                     np.broadcast_to(g2p, (128, D))]).astype(np.float32)
    betv = np.stack([np.broadcast_to(b1p, (128, D)),
                     np.broadcast_to(b2p, (128, D))]).astype(np.float32)
    in_maps = []
    for k in range(NCORES):
        in_maps.append({
            "xbuf": xb, "idxt": pp["idx_arr"][k], "dcol": pp["d_arr"][k],
            "wcol": pp["w_arr"][k], "bcol": pp["bloc_arr"][k],
            "iot": iotv, "idn": idnv, "onesr": onesv,
            "whi": whiv, "wlo": wlov, "brow": browv, "gam": gamv, "bet": betv,
        })
    return nc, in_maps, pp


def combine(pp, outs):
    sums = np.zeros((pp["G"] + PB, pp["D"]), dtype=np.float32)
    for k in range(NCORES):
        o = outs[k]
        for b in range(pp["NB"]):
            bb = pp["base"][k * pp["NB"] + b]
            sums[bb : bb + PB] += o[b * PB:(b + 1) * PB]
    return (sums[: pp["G"]]
            / np.maximum(pp["cnts"], 1.0)[:, None]).astype(np.float32)


def kernel(x, edge_index, batch, W1, b1, W2, b2,
           g1, beta1, rm1, rv1, g2, beta2, rm2, rv2):
    nc, in_maps, pp = prepare(x, edge_index, batch, W1, b1, W2, b2,
                              g1, beta1, rm1, rv1, g2, beta2, rm2, rv2)
    res = run_bass_kernel_spmd(nc, in_maps, core_ids=list(range(NCORES)))
    return combine(pp, [res.results[k]["outp"] for k in range(NCORES)])
